# revision 19
# baseline (speedup 1.0000x reference)
"""Trainium2 Bass kernel for a 3-layer relu-LSTM classifier.

Data-parallel over batch across 8 cores (B=16/core).  Single fused
software-pipelined wavefront: per 16-step chunk c the kernel runs

  SG : DMA-copy xT token slab for chunk c+2 into a ring (static APs)
  S0 : xg0 = xT @ W0 + b0 for chunk c+1           (PE bulk, N=256)
  S1 : L0 recurrence steps of chunk c
  S2 : xg1 = h0 @ W1 + b1 for chunk c-1           (PE bulk)
  S3 : L1 recurrence steps of chunk c-2
  S4 : xg2 = h1 @ W2 + b2 for chunk c-3           (PE bulk)
  S5 : L2 recurrence steps of chunk c-4

interleaved at per-timestep granularity, so each layer's elementwise
chain (1 ACT sigmoid + 4 DVE ops) hides under the other layers' matmul
streams.  All xg/h traffic stays in SBUF rings; nothing bounces through
DRAM.  Gates live in one PSUM tile [128, 4F] per step, order (f,i,o,cc),
injected via identity matmul.

Self-contained: hardcodes all shapes; host side only reformats weights.
"""

import os

import numpy as np
import ml_dtypes

BF16 = ml_dtypes.bfloat16
FP8 = ml_dtypes.float8_e4m3
U_FP8 = os.environ.get("K_UFP8", "1") == "1"

# Model dims
NCORES = 8
B_TOT, T = 128, 512
B = B_TOT // NCORES  # 16
VOCAB, EMB_D = 5000, 300
EMB_PAD = 384  # padded to 3*128
UNITS = [256, 512, 256]
DENSE = 64

# Wavefront geometry
TCr = 16             # timesteps per chunk
NCH = T // TCr       # 32 chunks
NTOK = T * B         # 8192 tokens per core
NBLK = NTOK // 128   # 64 gather blocks
CW = TCr * B         # 256 token columns per chunk

# Per-layer derived dims
LCFG = []
_d = EMB_PAD
for _u in UNITS:
    _nk = _u // 128
    LCFG.append(dict(u=_u, d=_d, nkw=_d // 128, nk=_nk, nm=4 * _nk,
                     F=_nk * 16, FW=4 * _nk * 16))
    _d = _u

STAGGERED = os.environ.get("K_STAGGERED", "1") == "1"

_CACHE = {}
LAST_RESULT = None  # BassKernelResults of the most recent run (for test.py)


def gate_perm(u):
    """Column permutation of [i f cc o]-ordered 4u gate dim into our
    m-tile order: blocks (f, i, o, cc), each block j-minor over u//128."""
    nk = u // 128
    base = [1, 0, 3, 2]  # keras gate idx (i=0, f=1, cc=2, o=3) -> (f, i, o, cc)
    perm = np.empty(4 * u, dtype=np.int64)
    for blk in range(4):
        for j in range(nk):
            m = blk * nk + j
            perm[m * 128:(m + 1) * 128] = base[blk] * u + j * 128 + np.arange(128)
    return perm


def fold_lhs(Wp, nkt, nm):
    """[nkt*128, nm*128] -> [128, nkt*nm*128] with tile (k, m) at cols
    ((k*nm)+m)*128."""
    K, M = Wp.shape
    assert K == nkt * 128 and M == nm * 128, (Wp.shape, nkt, nm)
    return np.ascontiguousarray(
        Wp.reshape(nkt, 128, nm, 128).transpose(1, 0, 2, 3).reshape(128, nkt * nm * 128)
    )


def prep_weights(inputs):
    """Host-side reformatting of the model weights (shared by all cores)."""
    f32 = lambda x: np.asarray(x, dtype=np.float32)
    out = {}
    perms = [gate_perm(u) for u in UNITS]
    W0 = np.zeros((EMB_PAD, 4 * UNITS[0]), np.float32)
    W0[:EMB_D] = f32(inputs["W0"])
    Ws = [W0, f32(inputs["W1"]), f32(inputs["W2"])]
    for l in range(3):
        cfg = LCFG[l]
        p = perms[l]
        out[f"w{l}"] = fold_lhs(Ws[l][:, p], cfg["nkw"], cfg["nm"]).astype(BF16)
        out[f"u{l}"] = fold_lhs(f32(inputs[f"U{l}"])[:, p], cfg["nk"], cfg["nm"]).astype(
            FP8 if U_FP8 else BF16)
        out[f"b{l}"] = np.ascontiguousarray(
            f32(inputs[f"b{l}"])[p].reshape(cfg["nm"], 128).T)
    Wd = f32(inputs["Wd"])  # [256, 64]
    out["wd"] = np.concatenate([Wd[0:128], Wd[128:256]], axis=1).astype(BF16)  # [128,128]
    out["bd"] = f32(inputs["bd"])           # [64]
    out["wc"] = f32(inputs["Wc"]).astype(BF16)  # [64, 1]
    out["bc"] = f32(inputs["bc"])           # [1]
    return out


def build_program():
    from concourse import bacc
    import concourse.mybir as mybir
    import concourse.tile as tile
    import concourse.bass as bass_mod
    from concourse.bass import ds
    from concourse.masks import make_identity

    FP32 = mybir.dt.float32
    BF = mybir.dt.bfloat16
    F8 = mybir.dt.float8e4
    UDT = F8 if U_FP8 else BF
    AF = mybir.ActivationFunctionType
    ALU = mybir.AluOpType

    nc = bacc.Bacc(None, target_bir_lowering=False)

    # ---- DRAM parameters ------------------------------------------------
    tok_d = nc.declare_dram_parameter("tokens_tb", [T * B], mybir.dt.int32, isOutput=False)
    emb_d = nc.declare_dram_parameter("emb", [VOCAB, EMB_D], FP32, isOutput=False)
    wp = {}
    for l in range(3):
        cfg = LCFG[l]
        wp[f"w{l}"] = nc.declare_dram_parameter(f"w{l}", [128, cfg["nkw"] * cfg["nm"] * 128], BF, isOutput=False)
        wp[f"u{l}"] = nc.declare_dram_parameter(f"u{l}", [128, cfg["nk"] * cfg["nm"] * 128], UDT, isOutput=False)
        wp[f"b{l}"] = nc.declare_dram_parameter(f"b{l}", [128, cfg["nm"]], FP32, isOutput=False)
    wd_d = nc.declare_dram_parameter("wd", [128, 128], BF, isOutput=False)
    bd_d = nc.declare_dram_parameter("bd", [DENSE], FP32, isOutput=False)
    wc_d = nc.declare_dram_parameter("wc", [DENSE, 1], BF, isOutput=False)
    bc_d = nc.declare_dram_parameter("bc", [1], FP32, isOutput=False)
    out_d = nc.declare_dram_parameter("out", [B], FP32, isOutput=True)

    F0, F1, F2 = LCFG[0]["F"], LCFG[1]["F"], LCFG[2]["F"]
    FW0, FW1, FW2 = LCFG[0]["FW"], LCFG[1]["FW"], LCFG[2]["FW"]

    with tile.TileContext(nc) as tc:
        def pool(name, bufs, space="SBUF"):
            return tc.tile_pool(name=name, bufs=bufs, space=space)

        with pool("const", 1) as constp, pool("wts", 1) as wtp, \
                pool("xT", 1) as xtp:
            ident = constp.tile([128, 128], FP32)
            make_identity(nc, ident[:])
            identb = constp.tile([128, 128], BF)
            make_identity(nc, identb[:])
            tok_sb = constp.tile([128, NBLK], mybir.dt.int32)
            nc.sync.dma_start(tok_sb[:], tok_d[:].rearrange("(i p) -> p i", p=128))
            bias_sb = []
            for l in range(3):
                bt = constp.tile([128, LCFG[l]["nm"]], FP32, tag=f"bias{l}")
                nc.sync.dma_start(bt[:], wp[f"b{l}"][:])
                bias_sb.append(bt)
            wd_sb = constp.tile([128, 128], BF)
            nc.sync.dma_start(wd_sb[:], wd_d[:])
            bd_sb = constp.tile([DENSE, 1], FP32)
            nc.sync.dma_start(bd_sb[:], bd_d[:])
            wc_sb = constp.tile([DENSE, 1], BF)
            nc.sync.dma_start(wc_sb[:], wc_d[:])
            bc_sb = constp.tile([1, 1], FP32)
            nc.sync.dma_start(bc_sb[:], bc_d[:])

            # weight tiles (resident)
            w_sb = {}
            for l in range(3):
                cfg = LCFG[l]
                wt = wtp.tile([128, cfg["nkw"] * cfg["nm"] * 128], BF, tag=f"w{l}")
                nc.sync.dma_start(wt[:], wp[f"w{l}"][:])
                w_sb[f"w{l}"] = wt
                ut = wtp.tile([128, cfg["nk"] * cfg["nm"] * 128], UDT, tag=f"u{l}")
                nc.sync.dma_start(ut[:], wp[f"u{l}"][:])
                w_sb[f"u{l}"] = ut

            # ============ xT: gathered+transposed tokens =============
            # xT layout: slab k in cols [k*NTOK, (k+1)*NTOK), token-major.
            # Filled chunk-by-chunk inside the wavefront (gather stage).
            xT = xtp.tile([128, 3 * NTOK], BF)
            nc.gpsimd.memset(xT[:, 2 * NTOK:3 * NTOK], 0.0)

            # ============ Wavefront =============
            with pool("rings", 1) as rp, pool("state", 1) as stp, \
                    pool("gath", 3) as gp, \
                    pool("tmp0", 2) as tp0, pool("tmp1", 2) as tp1, pool("tmp2", 2) as tp2:

                x0r = rp.tile([128, 2 * TCr * FW0], BF, tag="x0r")   # xg0 ring
                x1r = rp.tile([128, 2 * TCr * FW1], BF, tag="x1r")   # xg1 ring
                x2r = rp.tile([128, 2 * TCr * FW2], BF, tag="x2r")   # xg2 ring
                h0r = rp.tile([128, 2 * TCr * F0], BF, tag="h0r")    # h0 ring
                h1r = rp.tile([128, 2 * TCr * F1], BF, tag="h1r")    # h1 ring
                h2b = stp.tile([128, 2 * F2], BF, tag="h2b")         # h2 ping-pong
                cb0 = stp.tile([128, LCFG[0]["F"]], FP32, tag="cb0")
                cb1 = stp.tile([128, LCFG[1]["F"]], FP32, tag="cb1")
                cb2 = stp.tile([128, LCFG[2]["F"]], FP32, tag="cb2")
                cb = [cb0, cb1, cb2]
                warm = stp.tile([1, 1], FP32, tag="warm")

                # zero initial h/c state
                nc.gpsimd.memset(h0r[:, (TCr + TCr - 1) * F0:(2 * TCr) * F0], 0.0)
                nc.gpsimd.memset(h1r[:, (TCr + TCr - 1) * F1:(2 * TCr) * F1], 0.0)
                nc.gpsimd.memset(h2b[:, F2:2 * F2], 0.0)
                for l in range(3):
                    nc.gpsimd.memset(cb[l][:], 0.0)
                # hoist the sigmoid ACT table load out of the loop
                nc.scalar.activation(warm[:], cb[0][0:1, 0:1], AF.Sigmoid)

                _ps_cms = [pool("ps02", 2, "PSUM"), pool("ps1", 2, "PSUM"),
                           pool("tps", 2, "PSUM"), pool("bps", 2, "PSUM")]
                pp02, pp1, tpp, bpp = [p.__enter__() for p in _ps_cms]

                tmpp = [tp0, tp1, tp2]
                psp = [pp02, pp1, pp02]
                pstag = ["ps02", "ps1", "ps02"]
                hrings = [h0r, h1r, None]
                xgrings = [x0r, x1r, x2r]

                def rec_step(l, c, s):
                    """One LSTM step of layer l at (chunk c, step s)."""
                    cfg = LCFG[l]
                    F, FW, nk, nm = cfg["F"], cfg["FW"], cfg["nk"], cfg["nm"]
                    u_sb = w_sb[f"u{l}"]
                    slot = c % 2
                    xg = xgrings[l]
                    xg_sl = xg[:, (slot * TCr + s) * FW:(slot * TCr + s + 1) * FW]
                    if l < 2:
                        hr = hrings[l]
                        if s == 0:
                            h_prev = hr[:, (((c - 1) % 2) * TCr + TCr - 1) * F:
                                         (((c - 1) % 2) * TCr + TCr) * F]
                        else:
                            h_prev = hr[:, (slot * TCr + s - 1) * F:(slot * TCr + s) * F]
                        h_out = hr[:, (slot * TCr + s) * F:(slot * TCr + s + 1) * F]
                    else:
                        h_prev = h2b[:, ((s - 1) % 2) * F2:((s - 1) % 2 + 1) * F2]
                        h_out = h2b[:, (s % 2) * F2:(s % 2 + 1) * F2]

                    ps = psp[l].tile([128, FW], FP32, tag=pstag[l])
                    nc.tensor.matmul(ps[:], lhsT=identb[:], rhs=xg_sl,
                                     start=True, stop=False, skip_group_check=True)
                    for m in range(nm):
                        dst = ps[:, m * 16:(m + 1) * 16]
                        for k in range(nk):
                            nc.tensor.matmul(
                                dst, lhsT=u_sb[:, ((k * nm) + m) * 128:((k * nm) + m + 1) * 128],
                                rhs=h_prev[:, k * 16:(k + 1) * 16],
                                start=False, stop=(k == nk - 1), skip_group_check=True)
                    sfio = tmpp[l].tile([128, 3 * F], FP32, tag=f"sf{l}")
                    nc.scalar.activation(sfio[:], ps[:, 0:3 * F], AF.Sigmoid)
                    t1 = tmpp[l].tile([128, F], FP32, tag=f"t1{l}")
                    nc.vector.scalar_tensor_tensor(
                        out=t1[:], in0=ps[:, 3 * F:4 * F], scalar=0.0,
                        in1=sfio[:, F:2 * F], op0=ALU.max, op1=ALU.mult)
                    c2 = tmpp[l].tile([128, F], FP32, tag=f"c2{l}")
                    nc.vector.tensor_mul(out=c2[:], in0=cb[l][:], in1=sfio[:, 0:F])
                    nc.vector.tensor_add(out=cb[l][:], in0=c2[:], in1=t1[:])
                    nc.vector.scalar_tensor_tensor(
                        out=h_out, in0=cb[l][:], scalar=0.0,
                        in1=sfio[:, 2 * F:3 * F], op0=ALU.max, op1=ALU.mult)

                def proj_slice(pl, m, csrc):
                    """One m-tile of the xg{pl} chunk-projection for chunk csrc."""
                    cfg = LCFG[pl]
                    nkw, nm, FW = cfg["nkw"], cfg["nm"], cfg["FW"]
                    wt = w_sb[f"w{pl}"]
                    slot = csrc % 2
                    ps = bpp.tile([128, CW], FP32, tag="bps")
                    for k in range(nkw):
                        if pl == 0:
                            rhs = xT[:, k * NTOK + csrc * CW: k * NTOK + (csrc + 1) * CW]
                        else:
                            Fs = LCFG[pl - 1]["F"]
                            hsrc = hrings[pl - 1][:].rearrange("p (s w) -> p s w", w=Fs)
                            rhs = hsrc[:, slot * TCr:(slot + 1) * TCr, k * 16:(k + 1) * 16]
                        nc.tensor.matmul(ps[:], lhsT=wt[:, ((k * nm) + m) * 128:((k * nm) + m + 1) * 128],
                                         rhs=rhs, start=(k == 0), stop=(k == nkw - 1))
                    dst = xgrings[pl][:].rearrange("p (s w) -> p s w", w=FW)
                    nc.vector.tensor_scalar(
                        out=dst[:, slot * TCr:(slot + 1) * TCr, m * 16:(m + 1) * 16],
                        in0=ps[:].rearrange("p (s b) -> p s b", b=16),
                        scalar1=bias_sb[pl][:, m:m + 1], scalar2=None, op0=ALU.add)

                def gather_block(blk):
                    """Indirect-gather one 128-token block of embeddings."""
                    xb = gp.tile([128, EMB_PAD], FP32, tag="xb")
                    nc.gpsimd.indirect_dma_start(
                        out=xb[:, 0:EMB_D], out_offset=None,
                        in_=emb_d[:, :],
                        in_offset=bass_mod.IndirectOffsetOnAxis(
                            ap=tok_sb[:, blk:blk + 1], axis=0),
                    )
                    return xb

                def transpose_block(xb, blk, k):
                    """Transpose slab k of a gathered block into xT."""
                    tps = tpp.tile([128, 128], FP32, tag="tps")
                    nc.tensor.transpose(tps[:], xb[:, 128 * k:128 * (k + 1)], ident[:])
                    rows = 128 if k < 2 else 44
                    nc.vector.tensor_copy(
                        out=xT[0:rows, k * NTOK + 128 * blk: k * NTOK + 128 * (blk + 1)],
                        in_=tps[0:rows, :])

                def emit_master(c):
                    """One master-chunk position of the wavefront (full unroll)."""
                    g = c + 3 if c + 3 <= 31 else None
                    s0 = c + 1 <= 31
                    s1 = c <= 31
                    s2 = 0 <= c - 1 <= 31
                    s3 = 0 <= c - 2 <= 31
                    s4 = 0 <= c - 3 <= 31
                    s5 = 0 <= c - 4 <= 31
                    xb = {}
                    for s in range(TCr):
                        if g is not None:
                            if s == 0:
                                xb["a"] = gather_block(2 * g)
                            elif s in (2, 4, 6):
                                transpose_block(xb["a"], 2 * g, (s - 2) // 2)
                            elif s == 8:
                                xb["b"] = gather_block(2 * g + 1)
                            elif s in (10, 12, 14):
                                transpose_block(xb["b"], 2 * g + 1, (s - 10) // 2)
                        if s1:
                            rec_step(0, c, s)
                        if s0 and s % 2 == 0:
                            proj_slice(0, s // 2, c + 1)
                        if s2:
                            proj_slice(1, s, c - 1)
                        if s3:
                            rec_step(1, c - 2, s)
                        if s4 and s % 2 == 1:
                            proj_slice(2, (s - 1) // 2, c - 3)
                        if s5:
                            rec_step(2, c - 4, s)

                with nc.named_scope("wave"):
                    # pipeline fill: gather chunks 0..2, project xg0 chunk 0
                    for blk in range(6):
                        xb0 = gather_block(blk)
                        for k in range(3):
                            transpose_block(xb0, blk, k)
                    for s in range(0, TCr, 2):
                        proj_slice(0, s // 2, 0)
                    for c in range(36):
                        emit_master(c)

                for p in reversed(_ps_cms):
                    p.__exit__(None, None, None)

                # ============ dense head =============
                with nc.named_scope("dense"):
                    with pool("dps", 1, "PSUM") as dpp:
                        psd = dpp.tile([DENSE, 16], FP32, tag="psd")
                        for k in range(2):
                            nc.tensor.matmul(psd[:], lhsT=wd_sb[:, 64 * k:64 * (k + 1)],
                                             rhs=h2b[:, F2 + 16 * k:F2 + 16 * (k + 1)],
                                             start=(k == 0), stop=(k == 1))
                        hd = constp.tile([DENSE, 16], BF, tag="hd")
                        nc.scalar.activation(hd[:], psd[:], AF.Relu, bias=bd_sb[:, 0:1])
                        psc = dpp.tile([1, 16], FP32, tag="psc")
                        nc.tensor.matmul(psc[:], lhsT=wc_sb[:], rhs=hd[:], start=True, stop=True)
                        outv = constp.tile([1, 16], FP32, tag="outv")
                        nc.scalar.activation(outv[:], psc[:], AF.Sigmoid, bias=bc_sb[0:1, 0:1])
                        nc.sync.dma_start(out_d[:], outv[0:1, :])

    nc.finalize()
    return nc


def _get_program():
    if "nc" not in _CACHE:
        _CACHE["nc"] = build_program()
    return _CACHE["nc"]


def kernel(**inputs):
    global LAST_RESULT
    from concourse.bass_utils import run_bass_kernel_spmd

    nc = _get_program()
    w = prep_weights(inputs)
    tokens = np.asarray(inputs["tokens"], dtype=np.int32)  # [128, 512]

    in_maps = []
    for core in range(NCORES):
        tk = tokens[core * B:(core + 1) * B]          # [16, 512]
        tok_tb = np.ascontiguousarray(tk.T).reshape(-1)  # t-major: idx = t*16+b
        m = {"tokens_tb": tok_tb,
             "emb": np.asarray(inputs["emb"], dtype=np.float32)}
        m.update(w)
        in_maps.append(m)

    trace = os.environ.get("K_TRACE", "0") == "1"
    res = run_bass_kernel_spmd(nc, in_maps, list(range(NCORES)), trace=trace)
    LAST_RESULT = res
    out = np.concatenate([res.results[c]["out"].reshape(B, 1) for c in range(NCORES)], axis=0)
    return out.astype(np.float32)
